# revision 3
# baseline (speedup 1.0000x reference)
"""CBOW embedding-lookup kernel for Trainium2 (8 NeuronCores).

Math: out[b, o] = sum_i fc_w[o, i*V + contexts[b, i]] + fc_b[o]
i.e. a row-gather over a transposed view of the fc weight, summed over the
C=4 context slots, plus bias.

Strategy (BATCH_WAYS x VOCAB_WAYS = 8 cores):
  - Host: build table t[i, v, o] = fc_w[o, i*V+v] + fc_b[o]/C, shard o into
    VOCAB_WAYS contiguous column blocks -> per-core contiguous table
    [C*V, V/VOCAB_WAYS] f32. Folding bias/C into every row makes the device
    work a pure gather-accumulate: out row = sum of C gathered table rows.
  - Device: each core owns B/BATCH_WAYS batch rows and V/VOCAB_WAYS output
    cols. For each 128-row batch block, indirect-DMA-gather the C=4 rows per
    batch element with cce accumulate (bypass on slot 0, add on 1..3), then
    DMA the [128, V/VOCAB_WAYS] f32 block to the output.
  - Host: stitch the 8 per-core outputs into [B, V].
"""

import os

import numpy as np

from concourse import bacc, bass, mybir
import concourse.tile as tile
from concourse.bass_utils import run_bass_kernel_spmd

V = 8192          # vocab (both in and out)
C = 4             # context slots
B = 1024          # batch
M = 8             # cores
P = 128           # SBUF partitions / batch block
R = C * V         # table rows

BATCH_WAYS = int(os.environ.get("KERNEL_BATCH_WAYS", "1"))
VOCAB_WAYS = M // BATCH_WAYS
BS = B // BATCH_WAYS   # batch rows per core
VS = V // VOCAB_WAYS   # output cols per core
NBLK = BS // P         # 128-row batch blocks per core

_NC_CACHE = None
LAST_RESULTS = None  # test harness reads exec_time_ns from here


def _build_nc():
    nc = bacc.Bacc("TRN2", target_bir_lowering=False, debug=False)
    idx_d = nc.dram_tensor("idx", [BS, C], mybir.dt.int32, kind="ExternalInput")
    tab_d = nc.dram_tensor("tab", [R, VS], mybir.dt.float32, kind="ExternalInput")
    out_d = nc.dram_tensor("out", [BS, VS], mybir.dt.float32, kind="ExternalOutput")

    with tile.TileContext(nc) as tc:
        with tc.tile_pool(name="sbuf", bufs=min(NBLK, 8)) as pool:
            for blk in range(NBLK):
                row0 = blk * P
                idx_t = pool.tile([P, C], mybir.dt.int32, tag="idx")
                nc.sync.dma_start(out=idx_t[:], in_=idx_d[row0 : row0 + P, :])
                acc = pool.tile([P, VS], mybir.dt.float32, tag="acc")
                for i in range(C):
                    nc.gpsimd.indirect_dma_start(
                        out=acc[:],
                        out_offset=None,
                        in_=tab_d[:],
                        in_offset=bass.IndirectOffsetOnAxis(
                            ap=idx_t[:, i : i + 1], axis=0
                        ),
                        compute_op=(
                            mybir.AluOpType.bypass if i == 0 else mybir.AluOpType.add
                        ),
                    )
                nc.sync.dma_start(out=out_d[row0 : row0 + P, :], in_=acc[:])
    nc.compile()
    return nc


def _host_prep(contexts, fc_w, fc_b):
    idx = np.arange(C, dtype=np.int32)[None, :] * V + contexts.astype(np.int32)
    idx = np.ascontiguousarray(idx)

    w3 = fc_w.reshape(V, C, V)  # [o, i, v]
    bias_per_slot = (fc_b.astype(np.float32) / C)[:, None]  # [o, 1]
    vocab_shards = []
    for vw in range(VOCAB_WAYS):
        o_sl = slice(vw * VS, (vw + 1) * VS)
        shard = np.empty((C, V, VS), dtype=np.float32)
        for i in range(C):
            # [o_shard, v].T -> [v, o_shard], fused bias add
            np.add(w3[o_sl, i, :].T, bias_per_slot[o_sl].T, out=shard[i])
        vocab_shards.append(shard.reshape(R, VS))
    return idx, vocab_shards


def kernel(contexts, fc_w, fc_b):
    global _NC_CACHE, LAST_RESULTS
    idx, vocab_shards = _host_prep(contexts, fc_w, fc_b)
    if _NC_CACHE is None:
        _NC_CACHE = _build_nc()
    nc = _NC_CACHE

    # core m = bw * VOCAB_WAYS + vw owns batch rows [bw*BS:(bw+1)*BS] and
    # output cols [vw*VS:(vw+1)*VS]
    in_maps = []
    for m in range(M):
        bw, vw = divmod(m, VOCAB_WAYS)
        in_maps.append(
            {"idx": idx[bw * BS : (bw + 1) * BS], "tab": vocab_shards[vw]}
        )
    trace = bool(os.environ.get("KERNEL_TRACE"))
    res = run_bass_kernel_spmd(
        nc, in_maps, list(range(M)), trace=trace, stitch_traces=False
    )
    LAST_RESULTS = res

    out = np.empty((B, V), dtype=np.float32)
    for m in range(M):
        bw, vw = divmod(m, VOCAB_WAYS)
        out[bw * BS : (bw + 1) * BS, vw * VS : (vw + 1) * VS] = res.results[m]["out"]
    return out


# revision 5
# speedup vs baseline: 1.3858x; 1.3858x over previous
"""CBOW embedding-lookup kernel for Trainium2 (8 NeuronCores).

Math: out[b, o] = sum_i fc_w[o, i*V + contexts[b, i]] + fc_b[o]
i.e. a row-gather over a transposed view of the fc weight, summed over the
C=4 context slots, plus bias.

Strategy (BATCH_WAYS x VOCAB_WAYS = 8 cores):
  - Host: build table t[i, v, o] = fc_w[o, i*V+v] + fc_b[o]/C, shard o into
    VOCAB_WAYS contiguous column blocks -> per-core contiguous table
    [C*V, V/VOCAB_WAYS] f32. Folding bias/C into every row makes the device
    work a pure gather + 3 adds: out row = sum of C gathered table rows.
  - Device: each core owns B/BATCH_WAYS batch rows and V/VOCAB_WAYS output
    cols. For each 128-row batch block, a single indirect DMA gathers all
    C rows per batch element ([128, C] indices -> [128, C, VS] tile, one
    line-rate descriptor per row), then a chained DVE reduction sums the C
    slots, then the [128, VS] f32 block DMAs out.
  - Host: stitch the 8 per-core outputs into [B, V].

CCE-accumulate on the gather was measured 2x slower per descriptor than
bypass (SBUF read-modify-write), so the reduction runs on VectorE instead.
"""

import os

import numpy as np

from concourse import bacc, bass, mybir
import concourse.tile as tile
from concourse.bass_utils import run_bass_kernel_spmd

V = 8192          # vocab (both in and out)
C = 4             # context slots
B = 1024          # batch
M = 8             # cores
P = 128           # SBUF partitions / batch block
R = C * V         # table rows

BATCH_WAYS = int(os.environ.get("KERNEL_BATCH_WAYS", "2"))
VOCAB_WAYS = M // BATCH_WAYS
BS = B // BATCH_WAYS   # batch rows per core
VS = V // VOCAB_WAYS   # output cols per core
NBLK = BS // P         # 128-row batch blocks per core

_NC_CACHE = None
LAST_RESULTS = None  # test harness reads exec_time_ns from here


def _build_nc():
    nc = bacc.Bacc("TRN2", target_bir_lowering=False, debug=False)
    idx_d = nc.dram_tensor("idx", [BS, C], mybir.dt.int32, kind="ExternalInput")
    tab_d = nc.dram_tensor("tab", [R, VS], mybir.dt.float32, kind="ExternalInput")
    out_d = nc.dram_tensor("out", [BS, VS], mybir.dt.float32, kind="ExternalOutput")

    with tile.TileContext(nc) as tc:
        with tc.tile_pool(name="sbuf", bufs=min(NBLK, 3)) as pool:
            for blk in range(NBLK):
                row0 = blk * P
                idx_t = pool.tile([P, C], mybir.dt.int32, tag="idx")
                nc.sync.dma_start(out=idx_t[:], in_=idx_d[row0 : row0 + P, :])
                gath = pool.tile([P, C, VS], mybir.dt.float32, tag="gath")
                for i in range(C):
                    # NB: a multi-column offset AP ([P, C] indices in one op)
                    # passes CoreSim but returns garbage on HW — keep [P, 1].
                    nc.gpsimd.indirect_dma_start(
                        out=gath[:, i, :],
                        out_offset=None,
                        in_=tab_d[:],
                        in_offset=bass.IndirectOffsetOnAxis(
                            ap=idx_t[:, i : i + 1], axis=0
                        ),
                    )
                acc = pool.tile([P, VS], mybir.dt.float32, tag="acc")
                nc.vector.tensor_add(out=acc[:], in0=gath[:, 0, :], in1=gath[:, 1, :])
                nc.vector.tensor_add(out=acc[:], in0=acc[:], in1=gath[:, 2, :])
                nc.vector.tensor_add(out=acc[:], in0=acc[:], in1=gath[:, 3, :])
                nc.sync.dma_start(out=out_d[row0 : row0 + P, :], in_=acc[:])
    nc.compile()
    return nc


def _host_prep(contexts, fc_w, fc_b):
    idx = np.arange(C, dtype=np.int32)[None, :] * V + contexts.astype(np.int32)
    idx = np.ascontiguousarray(idx)

    w3 = fc_w.reshape(V, C, V)  # [o, i, v]
    bias_per_slot = (fc_b.astype(np.float32) / C)[:, None]  # [o, 1]
    vocab_shards = []
    for vw in range(VOCAB_WAYS):
        o_sl = slice(vw * VS, (vw + 1) * VS)
        shard = np.empty((C, V, VS), dtype=np.float32)
        for i in range(C):
            # [o_shard, v].T -> [v, o_shard], fused bias add
            np.add(w3[o_sl, i, :].T, bias_per_slot[o_sl].T, out=shard[i])
        vocab_shards.append(shard.reshape(R, VS))
    return idx, vocab_shards


def kernel(contexts, fc_w, fc_b):
    global _NC_CACHE, LAST_RESULTS
    idx, vocab_shards = _host_prep(contexts, fc_w, fc_b)
    if _NC_CACHE is None:
        _NC_CACHE = _build_nc()
    nc = _NC_CACHE

    # core m = bw * VOCAB_WAYS + vw owns batch rows [bw*BS:(bw+1)*BS] and
    # output cols [vw*VS:(vw+1)*VS]
    in_maps = []
    for m in range(M):
        bw, vw = divmod(m, VOCAB_WAYS)
        in_maps.append(
            {"idx": idx[bw * BS : (bw + 1) * BS], "tab": vocab_shards[vw]}
        )
    trace = bool(os.environ.get("KERNEL_TRACE"))
    res = run_bass_kernel_spmd(
        nc, in_maps, list(range(M)), trace=trace, stitch_traces=False
    )
    LAST_RESULTS = res

    out = np.empty((B, V), dtype=np.float32)
    for m in range(M):
        bw, vw = divmod(m, VOCAB_WAYS)
        out[bw * BS : (bw + 1) * BS, vw * VS : (vw + 1) * VS] = res.results[m]["out"]
    return out


# revision 7
# speedup vs baseline: 1.3925x; 1.0048x over previous
"""CBOW embedding-lookup kernel for Trainium2 (8 NeuronCores).

Math: out[b, o] = sum_i fc_w[o, i*V + contexts[b, i]] + fc_b[o]
i.e. a row-gather over a transposed view of the fc weight, summed over the
C=4 context slots, plus bias.

Strategy (BATCH_WAYS x VOCAB_WAYS = 8 cores):
  - Host: build table t[i, v, o] = fc_w[o, i*V+v] + fc_b[o]/C, shard o into
    VOCAB_WAYS contiguous column blocks -> per-core contiguous table
    [C*V, V/VOCAB_WAYS] f32. Folding bias/C into every row makes the device
    work a pure gather + 3 adds: out row = sum of C gathered table rows.
  - Device: each core owns B/BATCH_WAYS batch rows and V/VOCAB_WAYS output
    cols. For each 128-row batch block, a single indirect DMA gathers all
    C rows per batch element ([128, C] indices -> [128, C, VS] tile, one
    line-rate descriptor per row), then a chained DVE reduction sums the C
    slots, then the [128, VS] f32 block DMAs out.
  - Host: stitch the 8 per-core outputs into [B, V].

CCE-accumulate on the gather was measured 2x slower per descriptor than
bypass (SBUF read-modify-write), so the reduction runs on VectorE instead.
"""

import os

import numpy as np

from concourse import bacc, bass, mybir
import concourse.tile as tile
from concourse.bass_utils import run_bass_kernel_spmd

V = 8192          # vocab (both in and out)
C = 4             # context slots
B = 1024          # batch
M = 8             # cores
P = 128           # SBUF partitions / batch block
R = C * V         # table rows

BATCH_WAYS = int(os.environ.get("KERNEL_BATCH_WAYS", "2"))
VOCAB_WAYS = M // BATCH_WAYS
BS = B // BATCH_WAYS   # batch rows per core
VS = V // VOCAB_WAYS   # output cols per core
NBLK = BS // P         # 128-row batch blocks per core

_NC_CACHE = None
LAST_RESULTS = None  # test harness reads exec_time_ns from here


def _build_nc():
    nc = bacc.Bacc("TRN2", target_bir_lowering=False, debug=False)
    idx_d = nc.dram_tensor("idx", [BS, C], mybir.dt.int32, kind="ExternalInput")
    tab_d = nc.dram_tensor("tab", [R, VS], mybir.dt.float32, kind="ExternalInput")
    out_d = nc.dram_tensor("out", [BS, VS], mybir.dt.float32, kind="ExternalOutput")

    with tile.TileContext(nc) as tc:
        with tc.tile_pool(name="sbuf", bufs=1) as pool:
            idx_ts, gaths, accs = [], [], []
            for blk in range(NBLK):
                row0 = blk * P
                idx_t = pool.tile([P, C], mybir.dt.int32, tag=f"idx{blk}")
                nc.sync.dma_start(out=idx_t[:], in_=idx_d[row0 : row0 + P, :])
                idx_ts.append(idx_t)
                gaths.append(
                    pool.tile(
                        [P, C, VS], mybir.dt.float32, tag=f"g{blk}", name=f"g{blk}"
                    )
                )
                accs.append(
                    pool.tile([P, VS], mybir.dt.float32, tag=f"a{blk}", name=f"a{blk}")
                )
            # Slot-interleaved issue: each block's DVE add for slot i runs
            # while later gathers are still in flight, so only the last
            # block's final add sits in the tail.
            for i in range(C):
                for blk in range(NBLK):
                    # NB: a multi-column offset AP ([P, C] indices in one op)
                    # passes CoreSim but returns garbage on HW — keep [P, 1].
                    nc.gpsimd.indirect_dma_start(
                        out=gaths[blk][:, i, :],
                        out_offset=None,
                        in_=tab_d[:],
                        in_offset=bass.IndirectOffsetOnAxis(
                            ap=idx_ts[blk][:, i : i + 1], axis=0
                        ),
                    )
                if i >= 1:
                    for blk in range(NBLK):
                        nc.vector.tensor_add(
                            out=accs[blk][:],
                            in0=accs[blk][:] if i > 1 else gaths[blk][:, 0, :],
                            in1=gaths[blk][:, i, :],
                        )
            for blk in range(NBLK):
                row0 = blk * P
                nc.sync.dma_start(out=out_d[row0 : row0 + P, :], in_=accs[blk][:])
    nc.compile()
    return nc


def _host_prep(contexts, fc_w, fc_b):
    idx = np.arange(C, dtype=np.int32)[None, :] * V + contexts.astype(np.int32)
    idx = np.ascontiguousarray(idx)

    w3 = fc_w.reshape(V, C, V)  # [o, i, v]
    bias_per_slot = (fc_b.astype(np.float32) / C)[:, None]  # [o, 1]
    vocab_shards = []
    for vw in range(VOCAB_WAYS):
        o_sl = slice(vw * VS, (vw + 1) * VS)
        shard = np.empty((C, V, VS), dtype=np.float32)
        for i in range(C):
            # [o_shard, v].T -> [v, o_shard], fused bias add
            np.add(w3[o_sl, i, :].T, bias_per_slot[o_sl].T, out=shard[i])
        vocab_shards.append(shard.reshape(R, VS))
    return idx, vocab_shards


def kernel(contexts, fc_w, fc_b):
    global _NC_CACHE, LAST_RESULTS
    idx, vocab_shards = _host_prep(contexts, fc_w, fc_b)
    if _NC_CACHE is None:
        _NC_CACHE = _build_nc()
    nc = _NC_CACHE

    # core m = bw * VOCAB_WAYS + vw owns batch rows [bw*BS:(bw+1)*BS] and
    # output cols [vw*VS:(vw+1)*VS]
    in_maps = []
    for m in range(M):
        bw, vw = divmod(m, VOCAB_WAYS)
        in_maps.append(
            {"idx": idx[bw * BS : (bw + 1) * BS], "tab": vocab_shards[vw]}
        )
    trace = bool(os.environ.get("KERNEL_TRACE"))
    res = run_bass_kernel_spmd(
        nc, in_maps, list(range(M)), trace=trace, stitch_traces=False
    )
    LAST_RESULTS = res

    out = np.empty((B, V), dtype=np.float32)
    for m in range(M):
        bw, vw = divmod(m, VOCAB_WAYS)
        out[bw * BS : (bw + 1) * BS, vw * VS : (vw + 1) * VS] = res.results[m]["out"]
    return out


# revision 8
# speedup vs baseline: 1.6525x; 1.1867x over previous
"""CBOW embedding-lookup kernel for Trainium2 (8 NeuronCores).

Math: out[b, o] = sum_i fc_w[o, i*V + contexts[b, i]] + fc_b[o]
i.e. a row-gather over a transposed view of the fc weight, summed over the
C=4 context slots, plus bias.

Strategy (BATCH_WAYS x VOCAB_WAYS = 8 cores):
  - Host: build table t[i, v, o] = fc_w[o, i*V+v] + fc_b[o]/C, shard o into
    VOCAB_WAYS contiguous column blocks -> per-core contiguous table
    [C*V, V/VOCAB_WAYS] f32. Folding bias/C into every row makes the device
    work a pure gather + 3 adds: out row = sum of C gathered table rows.
  - Device: each core owns B/BATCH_WAYS batch rows and V/VOCAB_WAYS output
    cols. For each 128-row batch block, a single indirect DMA gathers all
    C rows per batch element ([128, C] indices -> [128, C, VS] tile, one
    line-rate descriptor per row), then a chained DVE reduction sums the C
    slots, then the [128, VS] f32 block DMAs out.
  - Host: stitch the 8 per-core outputs into [B, V].

CCE-accumulate on the gather was measured 2x slower per descriptor than
bypass (SBUF read-modify-write), so the reduction runs on VectorE instead.
"""

import os

import numpy as np

from concourse import bacc, bass, mybir
import concourse.tile as tile
from concourse.bass_utils import run_bass_kernel_spmd

V = 8192          # vocab (both in and out)
C = 4             # context slots
B = 1024          # batch
M = 8             # cores
P = 128           # SBUF partitions / batch block
R = C * V         # table rows

BATCH_WAYS = int(os.environ.get("KERNEL_BATCH_WAYS", "2"))
VOCAB_WAYS = M // BATCH_WAYS
BS = B // BATCH_WAYS   # batch rows per core
VS = V // VOCAB_WAYS   # output cols per core
NBLK = BS // P         # 128-row batch blocks per core

_NC_CACHE = None
LAST_RESULTS = None  # test harness reads exec_time_ns from here


def _build_nc():
    nc = bacc.Bacc("TRN2", target_bir_lowering=False, debug=False)
    idx_d = nc.dram_tensor("idx", [BS, C], mybir.dt.int32, kind="ExternalInput")
    tab_d = nc.dram_tensor("tab", [R, VS], mybir.dt.float32, kind="ExternalInput")
    out_d = nc.dram_tensor("out", [BS, VS], mybir.dt.float32, kind="ExternalOutput")

    with tile.TileContext(nc) as tc:
        with tc.tile_pool(name="sbuf", bufs=1) as pool:
            idx_ts, slots, accs = [], [], []
            for blk in range(NBLK):
                row0 = blk * P
                idx_t = pool.tile([P, C], mybir.dt.int32, tag=f"idx{blk}")
                nc.sync.dma_start(out=idx_t[:], in_=idx_d[row0 : row0 + P, :])
                idx_ts.append(idx_t)
                # one tile per (block, slot): no shared-tile WAR deps between
                # late gathers and the DVE reads of earlier slots
                slots.append(
                    [
                        pool.tile(
                            [P, VS],
                            mybir.dt.float32,
                            tag=f"g{blk}_{i}",
                            name=f"g{blk}_{i}",
                        )
                        for i in range(C)
                    ]
                )
                accs.append(
                    pool.tile([P, VS], mybir.dt.float32, tag=f"a{blk}", name=f"a{blk}")
                )

            def gather(blk, i):
                # NB: a multi-column offset AP ([P, C] indices in one op)
                # passes CoreSim but returns garbage on HW — keep [P, 1].
                nc.gpsimd.indirect_dma_start(
                    out=slots[blk][i][:],
                    out_offset=None,
                    in_=tab_d[:],
                    in_offset=bass.IndirectOffsetOnAxis(
                        ap=idx_ts[blk][:, i : i + 1], axis=0
                    ),
                )

            # Pair-first issue: slots 0+1 of each block stream in first so the
            # DVE reduction starts as early as possible; each later slot's add
            # chases its gather while other blocks' gathers keep the SDMA
            # engines saturated. Only the very last block's final add + store
            # sit in the tail.
            for blk in range(NBLK):
                gather(blk, 0)
                gather(blk, 1)
            for blk in range(NBLK):
                nc.vector.tensor_add(
                    out=accs[blk][:], in0=slots[blk][0][:], in1=slots[blk][1][:]
                )
            for i in range(2, C):
                for blk in range(NBLK):
                    gather(blk, i)
                for blk in range(NBLK):
                    nc.vector.tensor_add(
                        out=accs[blk][:], in0=accs[blk][:], in1=slots[blk][i][:]
                    )
            for blk in range(NBLK):
                row0 = blk * P
                nc.sync.dma_start(out=out_d[row0 : row0 + P, :], in_=accs[blk][:])
    nc.compile()
    return nc


def _host_prep(contexts, fc_w, fc_b):
    idx = np.arange(C, dtype=np.int32)[None, :] * V + contexts.astype(np.int32)
    idx = np.ascontiguousarray(idx)

    w3 = fc_w.reshape(V, C, V)  # [o, i, v]
    bias_per_slot = (fc_b.astype(np.float32) / C)[:, None]  # [o, 1]
    vocab_shards = []
    for vw in range(VOCAB_WAYS):
        o_sl = slice(vw * VS, (vw + 1) * VS)
        shard = np.empty((C, V, VS), dtype=np.float32)
        for i in range(C):
            # [o_shard, v].T -> [v, o_shard], fused bias add
            np.add(w3[o_sl, i, :].T, bias_per_slot[o_sl].T, out=shard[i])
        vocab_shards.append(shard.reshape(R, VS))
    return idx, vocab_shards


def kernel(contexts, fc_w, fc_b):
    global _NC_CACHE, LAST_RESULTS
    idx, vocab_shards = _host_prep(contexts, fc_w, fc_b)
    if _NC_CACHE is None:
        _NC_CACHE = _build_nc()
    nc = _NC_CACHE

    # core m = bw * VOCAB_WAYS + vw owns batch rows [bw*BS:(bw+1)*BS] and
    # output cols [vw*VS:(vw+1)*VS]
    in_maps = []
    for m in range(M):
        bw, vw = divmod(m, VOCAB_WAYS)
        in_maps.append(
            {"idx": idx[bw * BS : (bw + 1) * BS], "tab": vocab_shards[vw]}
        )
    trace = bool(os.environ.get("KERNEL_TRACE"))
    res = run_bass_kernel_spmd(
        nc, in_maps, list(range(M)), trace=trace, stitch_traces=False
    )
    LAST_RESULTS = res

    out = np.empty((B, V), dtype=np.float32)
    for m in range(M):
        bw, vw = divmod(m, VOCAB_WAYS)
        out[bw * BS : (bw + 1) * BS, vw * VS : (vw + 1) * VS] = res.results[m]["out"]
    return out
